# revision 18
# baseline (speedup 1.0000x reference)
"""GCN classifier (512 batched graphs x 200 nodes x 6400 edges) on 8 Trainium2 cores.

Data/graph parallel: 64 graphs per core, all edges graph-local.  Per graph a
dense 256x256 adjacency A^T (src-major, 32-wide src buckets x 64-wide dst
buckets) is accumulated on the TensorEngine from one-hot outer products.
One-hots are generated k-major (fine index as the middle axis, chunk as the
stride-1 last axis) so every DVE operand is a packed 2-byte stride-1 tensor,
unlocking the 2x_1P perf mode; a fraction of the one-hot work runs on GpSimd
in parallel.  All per-graph scalar math (degrees, normalizers) is batched
across the 64 graphs via PSUM column packing.  Layer 1 is a rank-2 matmul
relu(ac2 (x) w1 + invout (x) b1); layer 2 runs transposed ([dst, hid]
layout) with a rank-1 sq(deg) (x) b2 bias matmul so the mean readout becomes
a matmul against an inv_in/200 column, accumulating the [HID, 64] graph
readout directly in PSUM.  The MLP head and softmax are small fp16 matmuls
shared across the 64 graphs.  No collectives: the host concatenates the 8
per-core [64, 10] outputs.
"""

import sys

sys.path.insert(0, "/opt/trn_rl_repo")

import numpy as np

from concourse import bacc, bass, mybir, tile
from concourse.bass_utils import run_bass_kernel_spmd
from concourse.masks import make_identity

# Problem constants (hardcoded per the task contract).
N_GRAPHS = 512
NODES_PER_G = 200
EDGES_PER_G = 6400
E = N_GRAPHS * EDGES_PER_G
HID = 128
NCLS = 10
N_CORES = 8
GPC = N_GRAPHS // N_CORES          # graphs per core = 64
EPC = GPC * EDGES_PER_G            # edges per core
CHUNK = 128                        # edges per matmul chunk (K dim)
GROUP = 23                         # chunks per one-hot DVE instruction
POOL_EVERY = 10 ** 9               # every POOL_EVERY-th group's ohs on GpSimd
                                   # (walrus rejects TensorTensor on Pool)
PAD_IDX = 255                      # fine value marking a dead (padding) edge

F32 = mybir.dt.float32
F16 = mybir.dt.float16
I16 = mybir.dt.int16
RELU = mybir.ActivationFunctionType.Relu
SQRT = mybir.ActivationFunctionType.Sqrt
EXP = mybir.ActivationFunctionType.Exp

_PROGRAM_CACHE = {}
LAST_RESULTS = None  # BassKernelResults of the most recent run (for test.py)

MLP_DIMS = [(HID, 512), (512, 1024), (1024, 1024), (1024, 512), (512, NCLS)]


# --------------------------------------------------------------------------
# Host preprocessing: integer bucketing of edges into a uniform chunk schedule
# --------------------------------------------------------------------------

def _chunk_meta(C):
    """Chunk emission order: (sb, db, rank), round-robin over PE col strips."""
    C = np.asarray(C)
    meta = []
    for r in range(int(C.max())):
        for db in range(4):
            for sb in range(7):
                if r < C[sb, db]:
                    meta.append((sb, db, r))
    return meta


def _preprocess(src, dst):
    """Bucket each core's edges into (graph, srcb, dstb) chunks of 128.

    Returns (C, edata): C is the uniform 7x4 per-(srcb,dstb) chunk-count
    matrix (max over all 512 graphs, so every core runs the same program)
    and edata a list of per-core (sf, df) int16 array pairs, each
    [128, n_chunks], holding the fine src/dst index per edge slot; padding
    slots get PAD_IDX (matches nothing in the one-hot iota ranges).
    """
    src = np.asarray(src).astype(np.int64)
    dst = np.asarray(dst).astype(np.int64)
    g = np.arange(E, dtype=np.int64) // EDGES_PER_G
    src_l = src - g * NODES_PER_G
    dst_l = dst - g * NODES_PER_G
    assert src_l.min() >= 0 and src_l.max() < NODES_PER_G
    assert dst_l.min() >= 0 and dst_l.max() < NODES_PER_G

    srcb = src_l // 32                 # 0..6
    dstb = dst_l // 64                 # 0..3
    src_f = (src_l % 32).astype(np.int16)
    dst_f = (dst_l % 64).astype(np.int16)

    key_global = (g * 28 + srcb * 4 + dstb).astype(np.int64)
    cnt = np.bincount(key_global, minlength=N_GRAPHS * 28).reshape(N_GRAPHS, 7, 4)
    C = np.ceil(cnt.max(axis=0) / CHUNK).astype(np.int64)

    meta = _chunk_meta(C)
    chunks_pg = len(meta)
    maxC = int(C.max())
    slot_lut = np.full((7, 4, maxC), -1, dtype=np.int64)
    for i, (sb, db, r) in enumerate(meta):
        slot_lut[sb, db, r] = i
    cap_pg = chunks_pg * CHUNK

    edata = []
    for c in range(N_CORES):
        lo, hi = c * EPC, (c + 1) * EPC
        gl = g[lo:hi] - c * GPC
        key = gl * 28 + srcb[lo:hi] * 4 + dstb[lo:hi]
        order = np.argsort(key, kind="stable")
        k_sorted = key[order]
        grp_start = np.concatenate([[0], np.nonzero(np.diff(k_sorted))[0] + 1])
        starts = np.zeros(GPC * 28, dtype=np.int64)
        starts[k_sorted[grp_start]] = grp_start
        rank = np.arange(EPC) - starts[k_sorted]
        kg = k_sorted // 28
        ksb = (k_sorted % 28) // 4
        kdb = k_sorted % 4
        slot = (kg * cap_pg + slot_lut[ksb, kdb, rank // CHUNK] * CHUNK
                + rank % CHUNK)
        sf = np.full(GPC * cap_pg, PAD_IDX, dtype=np.int16)
        df = np.full(GPC * cap_pg, PAD_IDX, dtype=np.int16)
        sf[slot] = src_f[lo:hi][order]
        df[slot] = dst_f[lo:hi][order]
        # layout [n_slots] -> [n_chunks, 128] -> [128, n_chunks] (partition =
        # slot within chunk, free = chunk, contiguous along chunks)
        sf2 = np.ascontiguousarray(sf.reshape(-1, CHUNK).T)
        df2 = np.ascontiguousarray(df.reshape(-1, CHUNK).T)
        edata.append((sf2, df2))
    return tuple(map(tuple, C)), edata


# --------------------------------------------------------------------------
# Bass program
# --------------------------------------------------------------------------

def _build_program(C, stage=6):
    C = np.asarray(C)
    chunks_pg = int(C.sum())
    n_chunks = GPC * chunks_pg
    n_groups = (chunks_pg + GROUP - 1) // GROUP

    nc = bacc.Bacc(None, target_bir_lowering=False, debug=False)

    sf_d = nc.dram_tensor("sf", [128, 1, n_chunks], I16, kind="ExternalInput")
    df_d = nc.dram_tensor("df", [128, 1, n_chunks], I16, kind="ExternalInput")
    wrhs_d = nc.dram_tensor("wrhs", [128, HID], F16, kind="ExternalInput")
    w2_d = nc.dram_tensor("w2", [HID, HID], F16, kind="ExternalInput")
    dmask_d = nc.dram_tensor("dmask", [128, 2], F32, kind="ExternalInput")
    w_d, bc_d = [], []
    for li, (fi, fo) in enumerate(MLP_DIMS):
        w_d.append(nc.dram_tensor(f"mw{li}", [128, fi // 128, fo], F16,
                                  kind="ExternalInput"))
        bc_d.append(nc.dram_tensor(f"mbc{li}", [128, max(1, fo // 128)], F32,
                                   kind="ExternalInput"))
    out_d = nc.dram_tensor("out", [GPC, NCLS], F32, kind="ExternalOutput")
    dbg_d = (nc.dram_tensor("dbg", [128, 512], F32, kind="ExternalOutput")
             if stage < 6 else None)

    meta = _chunk_meta(C)
    # one accumulation epoch per src tile: zeroing matmul opens it
    # (start=True), the final chunk touching the tile closes it (stop=True)
    last_for_tile = {0: -1, 1: -1}
    for i, (sb, db, _) in enumerate(meta):
        last_for_tile[sb // 4] = i
    first_of_cell = [r == 0 for (_, _, r) in meta]

    dump = [None, 0, 0]

    def set_dump(ap, h, w):
        dump[0], dump[1], dump[2] = ap, h, w

    with tile.TileContext(nc) as tc:
        with (
            tc.tile_pool(name="glob", bufs=1) as gp,
            tc.tile_pool(name="oh", bufs=6) as ohp,
            tc.tile_pool(name="atps", bufs=2, space="PSUM") as atps,
            tc.tile_pool(name="colps", bufs=1, space="PSUM") as colps,
            tc.tile_pool(name="mmps", bufs=4, space="PSUM") as mmps,
            tc.tile_pool(name="work", bufs=2) as wp,
        ):
            # ---------------- constants / weights ----------------
            sf = gp.tile([128, 1, n_chunks], I16)
            nc.sync.dma_start(sf[:], sf_d[:])
            df = gp.tile([128, 1, n_chunks], I16)
            nc.sync.dma_start(df[:], df_d[:])

            # k-major iotas: value = fine index, constant along the chunk dim
            iota_s = gp.tile([128, 32, GROUP], I16)
            nc.gpsimd.iota(iota_s[:], pattern=[[1, 32], [0, GROUP]], base=0,
                           channel_multiplier=0)
            iota_t = gp.tile([128, 64, GROUP], I16)
            nc.gpsimd.iota(iota_t[:], pattern=[[1, 64], [0, GROUP]], base=0,
                           channel_multiplier=0)

            identf = gp.tile([128, 128], F32)
            make_identity(nc, identf[:])

            ones1 = gp.tile([128, 1], F16)
            nc.vector.memset(ones1[:], 1.0)
            zeros16 = gp.tile([128, 256], F16)
            nc.vector.memset(zeros16[:], 0.0)

            wrhs = gp.tile([128, HID], F16)
            nc.sync.dma_start(wrhs[:], wrhs_d[:])
            w2 = gp.tile([HID, HID], F16)
            nc.sync.dma_start(w2[:], w2_d[:])
            dmask = gp.tile([128, 2], F32)
            nc.sync.dma_start(dmask[:], dmask_d[:])
            w_sb, bc_sb = [], []
            for li, (fi, fo) in enumerate(MLP_DIMS):
                w = gp.tile([128, fi // 128, fo], F16)
                nc.sync.dma_start(w[:], w_d[li][:])
                w_sb.append(w)
                b = gp.tile([128, max(1, fo // 128)], F32)
                nc.sync.dma_start(b[:], bc_d[li][:])
                bc_sb.append(b)

            # ---------------- persistent per-graph state ----------------
            # A^T for all graphs: [128, g, st, 256] fp16 (src tile st: 0 ->
            # src slots 0..127, 1 -> 128..255; dst slots 0..255 on free)
            at_sb = gp.tile([128, GPC, 2, 256], F16)
            outd = gp.tile([128, 2, GPC], F32)
            s1 = gp.tile([128, 2, GPC], F16)
            # per-graph transpose feed: cols 0,1 = (ac2,invout) src tile 0;
            # 32,33 = src tile 1; 64 = sq dst-half 0; 96 = sq dst-half 1.
            # Transposing lands each operand group at a 32-aligned partition.
            pack = gp.tile([128, GPC, 128], F32)
            invw = gp.tile([128, 2, GPC], F16)      # invin/200, dst halves
            hg = gp.tile([128, 1, GPC], F16)        # readout [HID, g]

            # ---------------- pass A: adjacency + out-degrees -------------
            degt = colps.tile([128, 2, 2 * GPC], F32, tag="degt", name="degt")
            degps = [degt[:, 0, :], degt[:, 1, :]]
            for gi in range(GPC):
                at_ps = atps.tile([128, 2, 256], F32, tag="at", name="at_ps")
                for st in range(2):
                    nc.tensor.matmul(at_ps[:, st, :], lhsT=zeros16[:, 0:128],
                                     rhs=zeros16[:, 0:256], start=True,
                                     stop=False, skip_group_check=True)
                for grp in range(n_groups):
                    c0 = grp * GROUP
                    ng = min(GROUP, chunks_pg - c0)
                    cb = gi * chunks_pg + c0
                    ohs = ohp.tile([128, 32, GROUP], F16, tag="ohs", name="ohs")
                    ohd = ohp.tile([128, 64, GROUP], F16, tag="ohd", name="ohd")
                    eng_s = nc.gpsimd if (grp % POOL_EVERY == POOL_EVERY - 1) \
                        else nc.vector
                    eng_s.tensor_tensor(
                        out=ohs[:, :, 0:ng], in0=iota_s[:, :, 0:ng],
                        in1=sf[:, :, cb:cb + ng].to_broadcast([128, 32, ng]),
                        op=mybir.AluOpType.is_equal)
                    nc.vector.tensor_tensor(
                        out=ohd[:, :, 0:ng], in0=iota_t[:, :, 0:ng],
                        in1=df[:, :, cb:cb + ng].to_broadcast([128, 64, ng]),
                        op=mybir.AluOpType.is_equal)
                    for j in range(ng):
                        ci = c0 + j
                        sb, db, _ = meta[ci]
                        st, pb = sb // 4, (sb % 4) * 32
                        nc.tensor.matmul(
                            at_ps[pb:pb + 32, st, db * 64:(db + 1) * 64],
                            lhsT=ohs[:, :, j], rhs=ohd[:, :, j],
                            start=False, stop=(ci == last_for_tile[st]),
                            skip_group_check=True, tile_position=(0, pb))

                # copy A^T to SBUF fp16 on the scalar engine; accum_out
                # yields the free-axis row sums (= out-degrees) for free
                for st in range(2):
                    nc.scalar.activation(
                        at_sb[:, gi, st, :], at_ps[:, st, :],
                        mybir.ActivationFunctionType.Copy,
                        accum_out=outd[:, st, gi:gi + 1])
                set_dump(at_sb[:, gi, 0, :], 128, 256)
                # indeg: column sums via ones matvec, column-packed per graph
                if stage >= 3:
                    for dh in range(2):
                        for st in range(2):
                            nc.tensor.matmul(
                                degps[dh][:, 2 * gi:2 * gi + 1],
                                lhsT=at_sb[:, gi, st, dh * 128:(dh + 1) * 128],
                                rhs=ones1[:], start=(st == 0), stop=(st == 1),
                                skip_group_check=True)

            # ---------------- batch 1: invout ----------------
            if stage >= 2:
                invout = gp.tile([128, 2, GPC], F32)
                for st in range(2):
                    nc.vector.tensor_scalar(
                        out=invout[:, st, :], in0=outd[:, st, :], scalar1=1.0,
                        scalar2=None, op0=mybir.AluOpType.max)
                    nc.scalar.activation(invout[:, st, :], invout[:, st, :], SQRT)
                    nc.vector.reciprocal(invout[:, st, :], invout[:, st, :])
                set_dump(invout[:, 0, :], 128, GPC)

            # ---------------- batch 2: indeg -> invin, sq, invw, s1 -------
            if stage >= 3:
                indeg = gp.tile([128, 2, GPC], F32)
                invin = gp.tile([128, 2, GPC], F32)
                for dh in range(2):
                    nc.vector.tensor_copy(indeg[:, dh, :], degps[dh][:, 0::2])
                    nc.vector.tensor_scalar(
                        out=invin[:, dh, :], in0=indeg[:, dh, :], scalar1=1.0,
                        scalar2=None, op0=mybir.AluOpType.max)
                    nc.scalar.activation(invin[:, dh, :], invin[:, dh, :], SQRT)
                    nc.vector.tensor_copy(pack[:, :, 64 + 32 * dh],
                                          invin[:, dh, :])
                    nc.vector.reciprocal(invin[:, dh, :], invin[:, dh, :])
                    # dmask zeroes pad-node rows (dst slots 200..255, i.e.
                    # dst-half-1 partitions 72..127) so they never reach hg
                    nc.vector.tensor_scalar(
                        out=invw[:, dh, :], in0=invin[:, dh, :],
                        scalar1=dmask[:, dh:dh + 1], scalar2=1.0 / NODES_PER_G,
                        op0=mybir.AluOpType.mult, op1=mybir.AluOpType.mult)
                # s1 = indeg * invout (node n maps to the same partition in
                # dst-half dh and src-tile st layouts for n < 200)
                for st in range(2):
                    nc.vector.tensor_tensor(
                        out=s1[:, st, :], in0=indeg[:, st, :],
                        in1=invout[:, st, :], op=mybir.AluOpType.mult)
                set_dump(indeg[:, 0, :], 128, GPC)

            # ---------------- pass B: t1 columns, then ac2 ----------------
            if stage >= 4:
                for gi in range(GPC):
                    for dh in range(2):
                        for st in range(2):
                            nc.tensor.matmul(
                                degps[dh][:, 2 * gi + 1:2 * gi + 2],
                                lhsT=at_sb[:, gi, st, dh * 128:(dh + 1) * 128],
                                rhs=s1[:, st, gi:gi + 1],
                                start=(st == 0), stop=(st == 1),
                                skip_group_check=True)
                # ac2 = t1 * invin * invout -> pack col 32*st;
                # invout -> col 32*st + 1
                for st in range(2):
                    tmp = wp.tile([128, GPC], F32, tag=f"tmp{st}",
                                  name=f"acc_tmp{st}")
                    nc.vector.tensor_tensor(
                        out=tmp[:], in0=degps[st][:, 1::2],
                        in1=invin[:, st, :], op=mybir.AluOpType.mult)
                    nc.vector.tensor_tensor(
                        out=pack[:, :, 32 * st], in0=tmp[:],
                        in1=invout[:, st, :], op=mybir.AluOpType.mult)
                    nc.vector.tensor_copy(pack[:, :, 32 * st + 1],
                                          invout[:, st, :])
                set_dump(pack[:, :, 0], 128, GPC)

            # ------------ pass C: u, agg, h2^T, readout -------------------
            if stage >= 5:
                hgps = colps.tile([128, GPC], F32, tag="hgps", name="hgps")
                for gi in range(GPC):
                    # pack[g] -> rows: 0,1 / 32,33 = PQ^T per src tile;
                    # 64 / 96 = sq rows per dst half
                    pqt_ps = mmps.tile([128, 128], F32, tag="mm", name="pqt_ps")
                    nc.tensor.transpose(pqt_ps[:, :], pack[:, gi, :],
                                        identf[:])
                    pqt = wp.tile([128, 128], F16, tag="pqt", name="pqt")
                    nc.scalar.copy(pqt[:], pqt_ps[:])
                    # u = relu(ac2 (x) w1 + invout (x) b1)  [src, HID]
                    u = wp.tile([128, 2, HID], F16, tag="u", name="u")
                    for st in range(2):
                        ups = mmps.tile([128, HID], F32, tag="mm", name="ups")
                        nc.tensor.matmul(ups[:, :],
                                         lhsT=pqt[32 * st:32 * st + 2, :],
                                         rhs=wrhs[32 * st:32 * st + 2, :],
                                         start=True, stop=True,
                                         skip_group_check=True,
                                         tile_position=(32 * st, 0))
                        if st == 0:
                            nc.scalar.activation(u[:, st, :], ups[:, :], RELU)
                        else:
                            nc.vector.tensor_scalar(
                                out=u[:, st, :], in0=ups[:, :], scalar1=0.0,
                                scalar2=None, op0=mybir.AluOpType.max)
                    # aggT = u^T @ A^T  [HID, dst]
                    aggps_t = mmps.tile([128, 256], F32, tag="mm", name="aggps")
                    for st in range(2):
                        nc.tensor.matmul(aggps_t[:, :], lhsT=u[:, st, :],
                                         rhs=at_sb[:, gi, st, :],
                                         start=(st == 0), stop=(st == 1),
                                         skip_group_check=True)
                    aggT = wp.tile([128, 256], F16, tag="aggT", name="aggT")
                    nc.scalar.copy(aggT[:], aggps_t[:])
                    # h2^T[dst, HID] = aggT^T @ w2 + sq (x) b2, then relu
                    h2r = wp.tile([128, 2, HID], F16, tag="h2r", name="h2r")
                    for dh in range(2):
                        h2ps = mmps.tile([128, HID], F32, tag="mm", name="h2ps")
                        nc.tensor.matmul(h2ps[:, :],
                                         lhsT=aggT[:, dh * 128:(dh + 1) * 128],
                                         rhs=w2[:], start=True, stop=False,
                                         skip_group_check=True)
                        rb = 64 + 32 * dh
                        nc.tensor.matmul(h2ps[:, :],
                                         lhsT=pqt[rb:rb + 1, :],
                                         rhs=wrhs[rb:rb + 1, :],
                                         start=False, stop=True,
                                         skip_group_check=True,
                                         tile_position=(rb, 0))
                        if dh == 0:
                            nc.scalar.activation(h2r[:, dh, :], h2ps[:, :], RELU)
                        else:
                            nc.vector.tensor_scalar(
                                out=h2r[:, dh, :], in0=h2ps[:, :], scalar1=0.0,
                                scalar2=None, op0=mybir.AluOpType.max)
                    # readout: hg[:, gi] = sum_d h2r[d, :] * invin[d]/200
                    for dh in range(2):
                        nc.tensor.matmul(hgps[:, gi:gi + 1],
                                         lhsT=h2r[:, dh, :],
                                         rhs=invw[:, dh, gi:gi + 1],
                                         start=(dh == 0), stop=(dh == 1),
                                         skip_group_check=True)
                nc.vector.tensor_copy(hg[:, 0, :], hgps[:])
                set_dump(hg[:, 0, :], 128, GPC)

            if stage < 6 and dbg_d is not None:
                dbgt = gp.tile([128, 512], F32)
                nc.vector.memset(dbgt[:], 0.0)
                ap, h, w = dump
                w = min(512, w)
                nc.vector.tensor_copy(dbgt[0:h, 0:w], ap[:, 0:w])
                nc.sync.dma_start(dbg_d[:], dbgt[:])
                nc.sync.dma_start(out_d[:], dbgt[0:GPC, 0:NCLS])

            # ---------------- MLP head + softmax ----------------
            if stage >= 6:
                x = hg
                for li, (fi, fo) in enumerate(MLP_DIMS):
                    itiles = fi // 128
                    otiles = max(1, fo // 128)
                    m = 128 if fo >= 128 else fo
                    xn = gp.tile([128, otiles, GPC], F16, tag=f"x{li}",
                                 name=f"x{li}")
                    for ot in range(otiles):
                        ps = mmps.tile([128, GPC], F32, tag="mm", name="mlp_ps")
                        for it in range(itiles):
                            nc.tensor.matmul(
                                ps[0:m, 0:GPC],
                                lhsT=w_sb[li][:, it, ot * 128:ot * 128 + m],
                                rhs=x[:, it, :], start=(it == 0),
                                stop=(it == itiles - 1),
                                skip_group_check=True)
                        if li < len(MLP_DIMS) - 1:
                            if ot % 2 == 1:
                                nc.vector.tensor_scalar(
                                    out=xn[:, ot, :], in0=ps[:, 0:GPC],
                                    scalar1=bc_sb[li][:, ot:ot + 1],
                                    scalar2=0.0, op0=mybir.AluOpType.add,
                                    op1=mybir.AluOpType.max)
                            else:
                                nc.scalar.activation(
                                    xn[:, ot, :], ps[:, 0:GPC], RELU,
                                    bias=bc_sb[li][:, ot:ot + 1])
                        else:
                            nc.vector.tensor_scalar(
                                out=xn[0:m, ot, :], in0=ps[0:m, 0:GPC],
                                scalar1=bc_sb[li][0:m, ot:ot + 1], scalar2=None,
                                op0=mybir.AluOpType.add)
                    x = xn

                # softmax over classes: transpose [NCLS, GPC] -> [GPC, NCLS]
                x32 = gp.tile([NCLS, GPC], F32)
                nc.vector.tensor_copy(x32[:], x[0:NCLS, 0, :])
                tr_ps = mmps.tile([128, 128], F32, tag="mm", name="sm_ps")
                nc.tensor.transpose(tr_ps[0:GPC, 0:NCLS], x32[:],
                                    identf[0:NCLS, 0:NCLS])
                sm = gp.tile([GPC, NCLS], F32)
                nc.vector.tensor_copy(sm[:], tr_ps[0:GPC, 0:NCLS])
                mx = gp.tile([GPC, 1], F32)
                nc.vector.tensor_reduce(out=mx[:], in_=sm[:],
                                        axis=mybir.AxisListType.X,
                                        op=mybir.AluOpType.max)
                nc.vector.tensor_scalar(out=sm[:], in0=sm[:], scalar1=mx[:],
                                        scalar2=None,
                                        op0=mybir.AluOpType.subtract)
                nc.scalar.activation(sm[:], sm[:], EXP)
                ssum = gp.tile([GPC, 1], F32)
                nc.vector.tensor_reduce(out=ssum[:], in_=sm[:],
                                        axis=mybir.AxisListType.X,
                                        op=mybir.AluOpType.add)
                rsum = gp.tile([GPC, 1], F32)
                nc.vector.reciprocal(rsum[:], ssum[:])
                probs = gp.tile([GPC, NCLS], F32)
                nc.vector.tensor_scalar(out=probs[:], in0=sm[:], scalar1=rsum[:],
                                        scalar2=None, op0=mybir.AluOpType.mult)
                nc.sync.dma_start(out_d[:], probs[:])

    nc.compile()
    return nc


# --------------------------------------------------------------------------
# Entry point
# --------------------------------------------------------------------------

def _weights_inputs(W1, b1, W2, b2, Wa, ba, Wb, bb, Wc, bc, Wd, bd, We, be):
    wrhs = np.zeros((128, HID), np.float32)
    wrhs[0] = wrhs[32] = np.asarray(W1, np.float32).reshape(HID)
    wrhs[1] = wrhs[33] = np.asarray(b1, np.float32).reshape(HID)
    wrhs[64] = wrhs[96] = np.asarray(b2, np.float32).reshape(HID)
    base = {
        "wrhs": wrhs.astype(np.float16),
        "w2": np.ascontiguousarray(np.asarray(W2, np.float32)).astype(np.float16),
        "dmask": np.stack([np.ones(128, np.float32),
                           (np.arange(128) < 72).astype(np.float32)],
                          axis=1),
    }
    for li, (w, bvec) in enumerate(zip((Wa, Wb, Wc, Wd, We), (ba, bb, bc, bd, be))):
        w = np.asarray(w, np.float32)
        bvec = np.asarray(bvec, np.float32)
        fi, fo = w.shape
        base[f"mw{li}"] = np.ascontiguousarray(
            w.reshape(fi // 128, 128, fo).transpose(1, 0, 2)).astype(np.float16)
        if fo >= 128:
            bcol = np.ascontiguousarray(bvec.reshape(-1, 128).T)
        else:
            bcol = np.zeros((128, 1), np.float32)
            bcol[:fo, 0] = bvec
        base[f"mbc{li}"] = bcol
    return base


def kernel(src, dst, W1, b1, W2, b2, Wa, ba, Wb, bb, Wc, bc, Wd, bd, We, be,
           stage=6):
    global LAST_RESULTS
    C, edata = _preprocess(src, dst)
    key = (C, stage)
    if key not in _PROGRAM_CACHE:
        _PROGRAM_CACHE[key] = _build_program(C, stage=stage)
    nc = _PROGRAM_CACHE[key]

    base = _weights_inputs(W1, b1, W2, b2, Wa, ba, Wb, bb, Wc, bc, Wd, bd,
                           We, be)
    in_maps = [dict(base, sf=edata[c][0].reshape(128, 1, -1),
                    df=edata[c][1].reshape(128, 1, -1))
               for c in range(N_CORES)]
    LAST_RESULTS = run_bass_kernel_spmd(nc, in_maps, list(range(N_CORES)))
    out = np.concatenate([LAST_RESULTS.results[c]["out"] for c in range(N_CORES)],
                         axis=0)
    return out.astype(np.float32)


# revision 19
# speedup vs baseline: 1.0823x; 1.0823x over previous
"""GCN classifier (512 batched graphs x 200 nodes x 6400 edges) on 8 Trainium2 cores.

Data/graph parallel: 64 graphs per core, all edges graph-local.  Per graph a
dense 256x256 adjacency A^T (src-major, 32-wide src buckets x 64-wide dst
buckets) is accumulated on the TensorEngine from one-hot outer products.
One-hots are generated k-major (fine index as the middle axis, chunk as the
stride-1 last axis) so every DVE operand is a packed 2-byte stride-1 tensor,
unlocking the 2x_1P perf mode; a fraction of the one-hot work runs on GpSimd
in parallel.  All per-graph scalar math (degrees, normalizers) is batched
across the 64 graphs via PSUM column packing.  Layer 1 is a rank-2 matmul
relu(ac2 (x) w1 + invout (x) b1); layer 2 runs transposed ([dst, hid]
layout) with a rank-1 sq(deg) (x) b2 bias matmul so the mean readout becomes
a matmul against an inv_in/200 column, accumulating the [HID, 64] graph
readout directly in PSUM.  The MLP head and softmax are small fp16 matmuls
shared across the 64 graphs.  No collectives: the host concatenates the 8
per-core [64, 10] outputs.
"""

import sys

sys.path.insert(0, "/opt/trn_rl_repo")

import numpy as np

from concourse import bacc, bass, mybir, tile
from concourse.bass_utils import run_bass_kernel_spmd
from concourse.masks import make_identity

# Problem constants (hardcoded per the task contract).
N_GRAPHS = 512
NODES_PER_G = 200
EDGES_PER_G = 6400
E = N_GRAPHS * EDGES_PER_G
HID = 128
NCLS = 10
N_CORES = 8
GPC = N_GRAPHS // N_CORES          # graphs per core = 64
EPC = GPC * EDGES_PER_G            # edges per core
CHUNK = 128                        # edges per matmul chunk (K dim)
GROUP = 69                         # chunks per one-hot DVE instruction
POOL_EVERY = 10 ** 9               # every POOL_EVERY-th group's ohs on GpSimd
                                   # (walrus rejects TensorTensor on Pool)
PAD_IDX = 255                      # fine value marking a dead (padding) edge

F32 = mybir.dt.float32
F16 = mybir.dt.float16
I16 = mybir.dt.int16
RELU = mybir.ActivationFunctionType.Relu
SQRT = mybir.ActivationFunctionType.Sqrt
EXP = mybir.ActivationFunctionType.Exp

_PROGRAM_CACHE = {}
LAST_RESULTS = None  # BassKernelResults of the most recent run (for test.py)

MLP_DIMS = [(HID, 512), (512, 1024), (1024, 1024), (1024, 512), (512, NCLS)]


# --------------------------------------------------------------------------
# Host preprocessing: integer bucketing of edges into a uniform chunk schedule
# --------------------------------------------------------------------------

def _chunk_meta(C):
    """Chunk emission order: (sb, db, rank), round-robin over PE col strips."""
    C = np.asarray(C)
    meta = []
    for r in range(int(C.max())):
        for db in range(4):
            for sb in range(7):
                if r < C[sb, db]:
                    meta.append((sb, db, r))
    return meta


def _preprocess(src, dst):
    """Bucket each core's edges into (graph, srcb, dstb) chunks of 128.

    Returns (C, edata): C is the uniform 7x4 per-(srcb,dstb) chunk-count
    matrix (max over all 512 graphs, so every core runs the same program)
    and edata a list of per-core (sf, df) int16 array pairs, each
    [128, n_chunks], holding the fine src/dst index per edge slot; padding
    slots get PAD_IDX (matches nothing in the one-hot iota ranges).
    """
    src = np.asarray(src).astype(np.int64)
    dst = np.asarray(dst).astype(np.int64)
    g = np.arange(E, dtype=np.int64) // EDGES_PER_G
    src_l = src - g * NODES_PER_G
    dst_l = dst - g * NODES_PER_G
    assert src_l.min() >= 0 and src_l.max() < NODES_PER_G
    assert dst_l.min() >= 0 and dst_l.max() < NODES_PER_G

    srcb = src_l // 32                 # 0..6
    dstb = dst_l // 64                 # 0..3
    src_f = (src_l % 32).astype(np.int16)
    dst_f = (dst_l % 64).astype(np.int16)

    key_global = (g * 28 + srcb * 4 + dstb).astype(np.int64)
    cnt = np.bincount(key_global, minlength=N_GRAPHS * 28).reshape(N_GRAPHS, 7, 4)
    C = np.ceil(cnt.max(axis=0) / CHUNK).astype(np.int64)

    meta = _chunk_meta(C)
    chunks_pg = len(meta)
    maxC = int(C.max())
    slot_lut = np.full((7, 4, maxC), -1, dtype=np.int64)
    for i, (sb, db, r) in enumerate(meta):
        slot_lut[sb, db, r] = i
    cap_pg = chunks_pg * CHUNK

    edata = []
    for c in range(N_CORES):
        lo, hi = c * EPC, (c + 1) * EPC
        gl = g[lo:hi] - c * GPC
        key = gl * 28 + srcb[lo:hi] * 4 + dstb[lo:hi]
        order = np.argsort(key, kind="stable")
        k_sorted = key[order]
        grp_start = np.concatenate([[0], np.nonzero(np.diff(k_sorted))[0] + 1])
        starts = np.zeros(GPC * 28, dtype=np.int64)
        starts[k_sorted[grp_start]] = grp_start
        rank = np.arange(EPC) - starts[k_sorted]
        kg = k_sorted // 28
        ksb = (k_sorted % 28) // 4
        kdb = k_sorted % 4
        slot = (kg * cap_pg + slot_lut[ksb, kdb, rank // CHUNK] * CHUNK
                + rank % CHUNK)
        sf = np.full(GPC * cap_pg, PAD_IDX, dtype=np.int16)
        df = np.full(GPC * cap_pg, PAD_IDX, dtype=np.int16)
        sf[slot] = src_f[lo:hi][order]
        df[slot] = dst_f[lo:hi][order]
        # layout [n_slots] -> [n_chunks, 128] -> [128, n_chunks] (partition =
        # slot within chunk, free = chunk, contiguous along chunks)
        sf2 = np.ascontiguousarray(sf.reshape(-1, CHUNK).T)
        df2 = np.ascontiguousarray(df.reshape(-1, CHUNK).T)
        edata.append((sf2, df2))
    return tuple(map(tuple, C)), edata


# --------------------------------------------------------------------------
# Bass program
# --------------------------------------------------------------------------

def _build_program(C, stage=6):
    C = np.asarray(C)
    chunks_pg = int(C.sum())
    n_chunks = GPC * chunks_pg
    n_groups = (chunks_pg + GROUP - 1) // GROUP

    nc = bacc.Bacc(None, target_bir_lowering=False, debug=False)

    sf_d = nc.dram_tensor("sf", [128, 1, n_chunks], I16, kind="ExternalInput")
    df_d = nc.dram_tensor("df", [128, 1, n_chunks], I16, kind="ExternalInput")
    wrhs_d = nc.dram_tensor("wrhs", [128, HID], F16, kind="ExternalInput")
    w2_d = nc.dram_tensor("w2", [HID, HID], F16, kind="ExternalInput")
    dmask_d = nc.dram_tensor("dmask", [128, 2], F32, kind="ExternalInput")
    w_d, bc_d = [], []
    for li, (fi, fo) in enumerate(MLP_DIMS):
        w_d.append(nc.dram_tensor(f"mw{li}", [128, fi // 128, fo], F16,
                                  kind="ExternalInput"))
        bc_d.append(nc.dram_tensor(f"mbc{li}", [128, max(1, fo // 128)], F32,
                                   kind="ExternalInput"))
    out_d = nc.dram_tensor("out", [GPC, NCLS], F32, kind="ExternalOutput")
    dbg_d = (nc.dram_tensor("dbg", [128, 512], F32, kind="ExternalOutput")
             if stage < 6 else None)

    meta = _chunk_meta(C)
    # one accumulation epoch per src tile: zeroing matmul opens it
    # (start=True), the final chunk touching the tile closes it (stop=True)
    last_for_tile = {0: -1, 1: -1}
    for i, (sb, db, _) in enumerate(meta):
        last_for_tile[sb // 4] = i
    first_of_cell = [r == 0 for (_, _, r) in meta]

    dump = [None, 0, 0]

    def set_dump(ap, h, w):
        dump[0], dump[1], dump[2] = ap, h, w

    with tile.TileContext(nc) as tc:
        with (
            tc.tile_pool(name="glob", bufs=1) as gp,
            tc.tile_pool(name="oh", bufs=2) as ohp,
            tc.tile_pool(name="atps", bufs=2, space="PSUM") as atps,
            tc.tile_pool(name="colps", bufs=1, space="PSUM") as colps,
            tc.tile_pool(name="mmps", bufs=4, space="PSUM") as mmps,
            tc.tile_pool(name="work", bufs=3) as wp,
        ):
            # ---------------- constants / weights ----------------
            sf = gp.tile([128, 1, n_chunks], I16)
            nc.sync.dma_start(sf[:], sf_d[:])
            df = gp.tile([128, 1, n_chunks], I16)
            nc.sync.dma_start(df[:], df_d[:])

            # k-major iotas: value = fine index, constant along the chunk dim
            iota_s = gp.tile([128, 32, GROUP], I16)
            nc.gpsimd.iota(iota_s[:], pattern=[[1, 32], [0, GROUP]], base=0,
                           channel_multiplier=0)
            iota_t = gp.tile([128, 64, GROUP], I16)
            nc.gpsimd.iota(iota_t[:], pattern=[[1, 64], [0, GROUP]], base=0,
                           channel_multiplier=0)

            identf = gp.tile([128, 128], F32)
            make_identity(nc, identf[:])

            ones1 = gp.tile([128, 1], F16)
            nc.vector.memset(ones1[:], 1.0)
            zeros16 = gp.tile([128, 256], F16)
            nc.vector.memset(zeros16[:], 0.0)

            wrhs = gp.tile([128, HID], F16)
            nc.sync.dma_start(wrhs[:], wrhs_d[:])
            w2 = gp.tile([HID, HID], F16)
            nc.sync.dma_start(w2[:], w2_d[:])
            dmask = gp.tile([128, 2], F32)
            nc.sync.dma_start(dmask[:], dmask_d[:])
            w_sb, bc_sb = [], []
            for li, (fi, fo) in enumerate(MLP_DIMS):
                w = gp.tile([128, fi // 128, fo], F16)
                nc.sync.dma_start(w[:], w_d[li][:])
                w_sb.append(w)
                b = gp.tile([128, max(1, fo // 128)], F32)
                nc.sync.dma_start(b[:], bc_d[li][:])
                bc_sb.append(b)

            # ---------------- persistent per-graph state ----------------
            # A^T for all graphs: [128, g, st, 256] fp16 (src tile st: 0 ->
            # src slots 0..127, 1 -> 128..255; dst slots 0..255 on free)
            at_sb = gp.tile([128, GPC, 2, 256], F16)
            outd = gp.tile([128, 2, GPC], F32)
            s1 = gp.tile([128, 2, GPC], F16)
            # per-graph transpose feed: cols 0,1 = (ac2,invout) src tile 0;
            # 32,33 = src tile 1; 64 = sq dst-half 0; 96 = sq dst-half 1.
            # Transposing lands each operand group at a 32-aligned partition.
            pack = gp.tile([128, GPC, 128], F32)
            invw = gp.tile([128, 2, GPC], F16)      # invin/200, dst halves
            hg = gp.tile([128, 1, GPC], F16)        # readout [HID, g]

            # ---------------- pass A: adjacency + out-degrees -------------
            degt = colps.tile([128, 2, 2 * GPC], F32, tag="degt", name="degt")
            degps = [degt[:, 0, :], degt[:, 1, :]]
            for gi in range(GPC):
                at_ps = atps.tile([128, 2, 256], F32, tag="at", name="at_ps")
                for st in range(2):
                    nc.tensor.matmul(at_ps[:, st, :], lhsT=zeros16[:, 0:128],
                                     rhs=zeros16[:, 0:256], start=True,
                                     stop=False, skip_group_check=True)
                for grp in range(n_groups):
                    c0 = grp * GROUP
                    ng = min(GROUP, chunks_pg - c0)
                    cb = gi * chunks_pg + c0
                    ohs = ohp.tile([128, 32, GROUP], F16, tag="ohs", name="ohs")
                    ohd = ohp.tile([128, 64, GROUP], F16, tag="ohd", name="ohd")
                    eng_s = nc.gpsimd if (grp % POOL_EVERY == POOL_EVERY - 1) \
                        else nc.vector
                    eng_s.tensor_tensor(
                        out=ohs[:, :, 0:ng], in0=iota_s[:, :, 0:ng],
                        in1=sf[:, :, cb:cb + ng].to_broadcast([128, 32, ng]),
                        op=mybir.AluOpType.is_equal)
                    nc.vector.tensor_tensor(
                        out=ohd[:, :, 0:ng], in0=iota_t[:, :, 0:ng],
                        in1=df[:, :, cb:cb + ng].to_broadcast([128, 64, ng]),
                        op=mybir.AluOpType.is_equal)
                    for j in range(ng):
                        ci = c0 + j
                        sb, db, _ = meta[ci]
                        st, pb = sb // 4, (sb % 4) * 32
                        nc.tensor.matmul(
                            at_ps[pb:pb + 32, st, db * 64:(db + 1) * 64],
                            lhsT=ohs[:, :, j], rhs=ohd[:, :, j],
                            start=False, stop=(ci == last_for_tile[st]),
                            skip_group_check=True, tile_position=(0, pb))

                # copy A^T to SBUF fp16 on the scalar engine; accum_out
                # yields the free-axis row sums (= out-degrees) for free
                for st in range(2):
                    nc.scalar.activation(
                        at_sb[:, gi, st, :], at_ps[:, st, :],
                        mybir.ActivationFunctionType.Copy,
                        accum_out=outd[:, st, gi:gi + 1])
                set_dump(at_sb[:, gi, 0, :], 128, 256)
                # indeg: column sums via ones matvec, column-packed per graph
                if stage >= 3:
                    for dh in range(2):
                        for st in range(2):
                            nc.tensor.matmul(
                                degps[dh][:, 2 * gi:2 * gi + 1],
                                lhsT=at_sb[:, gi, st, dh * 128:(dh + 1) * 128],
                                rhs=ones1[:], start=(st == 0), stop=(st == 1),
                                skip_group_check=True)

            # ---------------- batch 1: invout ----------------
            if stage >= 2:
                invout = gp.tile([128, 2, GPC], F32)
                for st in range(2):
                    nc.vector.tensor_scalar(
                        out=invout[:, st, :], in0=outd[:, st, :], scalar1=1.0,
                        scalar2=None, op0=mybir.AluOpType.max)
                    nc.scalar.activation(invout[:, st, :], invout[:, st, :], SQRT)
                    nc.vector.reciprocal(invout[:, st, :], invout[:, st, :])
                set_dump(invout[:, 0, :], 128, GPC)

            # ---------------- batch 2: indeg -> invin, sq, invw, s1 -------
            if stage >= 3:
                indeg = gp.tile([128, 2, GPC], F32)
                invin = gp.tile([128, 2, GPC], F32)
                for dh in range(2):
                    nc.vector.tensor_copy(indeg[:, dh, :], degps[dh][:, 0::2])
                    nc.vector.tensor_scalar(
                        out=invin[:, dh, :], in0=indeg[:, dh, :], scalar1=1.0,
                        scalar2=None, op0=mybir.AluOpType.max)
                    nc.scalar.activation(invin[:, dh, :], invin[:, dh, :], SQRT)
                    nc.vector.tensor_copy(pack[:, :, 64 + 32 * dh],
                                          invin[:, dh, :])
                    nc.vector.reciprocal(invin[:, dh, :], invin[:, dh, :])
                    # dmask zeroes pad-node rows (dst slots 200..255, i.e.
                    # dst-half-1 partitions 72..127) so they never reach hg
                    nc.vector.tensor_scalar(
                        out=invw[:, dh, :], in0=invin[:, dh, :],
                        scalar1=dmask[:, dh:dh + 1], scalar2=1.0 / NODES_PER_G,
                        op0=mybir.AluOpType.mult, op1=mybir.AluOpType.mult)
                # s1 = indeg * invout (node n maps to the same partition in
                # dst-half dh and src-tile st layouts for n < 200)
                for st in range(2):
                    nc.vector.tensor_tensor(
                        out=s1[:, st, :], in0=indeg[:, st, :],
                        in1=invout[:, st, :], op=mybir.AluOpType.mult)
                set_dump(indeg[:, 0, :], 128, GPC)

            # ---------------- pass B: t1 columns, then ac2 ----------------
            if stage >= 4:
                for gi in range(GPC):
                    for dh in range(2):
                        for st in range(2):
                            nc.tensor.matmul(
                                degps[dh][:, 2 * gi + 1:2 * gi + 2],
                                lhsT=at_sb[:, gi, st, dh * 128:(dh + 1) * 128],
                                rhs=s1[:, st, gi:gi + 1],
                                start=(st == 0), stop=(st == 1),
                                skip_group_check=True)
                # ac2 = t1 * invin * invout -> pack col 32*st;
                # invout -> col 32*st + 1
                for st in range(2):
                    tmp = wp.tile([128, GPC], F32, tag=f"tmp{st}",
                                  name=f"acc_tmp{st}")
                    nc.vector.tensor_tensor(
                        out=tmp[:], in0=degps[st][:, 1::2],
                        in1=invin[:, st, :], op=mybir.AluOpType.mult)
                    nc.vector.tensor_tensor(
                        out=pack[:, :, 32 * st], in0=tmp[:],
                        in1=invout[:, st, :], op=mybir.AluOpType.mult)
                    nc.vector.tensor_copy(pack[:, :, 32 * st + 1],
                                          invout[:, st, :])
                set_dump(pack[:, :, 0], 128, GPC)

            # ------------ pass C: u, agg, h2^T, readout -------------------
            if stage >= 5:
                hgps = colps.tile([128, GPC], F32, tag="hgps", name="hgps")
                for gi in range(GPC):
                    # pack[g] -> rows: 0,1 / 32,33 = PQ^T per src tile;
                    # 64 / 96 = sq rows per dst half
                    pqt_ps = mmps.tile([128, 128], F32, tag="mm", name="pqt_ps")
                    nc.tensor.transpose(pqt_ps[:, :], pack[:, gi, :],
                                        identf[:])
                    pqt = wp.tile([128, 128], F16, tag="pqt", name="pqt")
                    nc.scalar.copy(pqt[:], pqt_ps[:])
                    # u = relu(ac2 (x) w1 + invout (x) b1)  [src, HID]
                    u = wp.tile([128, 2, HID], F16, tag="u", name="u")
                    for st in range(2):
                        ups = mmps.tile([128, HID], F32, tag="mm", name="ups")
                        nc.tensor.matmul(ups[:, :],
                                         lhsT=pqt[32 * st:32 * st + 2, :],
                                         rhs=wrhs[32 * st:32 * st + 2, :],
                                         start=True, stop=True,
                                         skip_group_check=True,
                                         tile_position=(32 * st, 0))
                        if st == 0:
                            nc.scalar.activation(u[:, st, :], ups[:, :], RELU)
                        else:
                            nc.vector.tensor_scalar(
                                out=u[:, st, :], in0=ups[:, :], scalar1=0.0,
                                scalar2=None, op0=mybir.AluOpType.max)
                    # aggT = u^T @ A^T  [HID, dst]
                    aggps_t = mmps.tile([128, 256], F32, tag="mm", name="aggps")
                    for st in range(2):
                        nc.tensor.matmul(aggps_t[:, :], lhsT=u[:, st, :],
                                         rhs=at_sb[:, gi, st, :],
                                         start=(st == 0), stop=(st == 1),
                                         skip_group_check=True)
                    aggT = wp.tile([128, 256], F16, tag="aggT", name="aggT")
                    nc.vector.tensor_copy(aggT[:, 0:128], aggps_t[:, 0:128])
                    nc.scalar.copy(aggT[:, 128:256], aggps_t[:, 128:256])
                    # h2^T[dst, HID] = aggT^T @ w2 + sq (x) b2, then relu
                    h2r = wp.tile([128, 2, HID], F16, tag="h2r", name="h2r")
                    for dh in range(2):
                        h2ps = mmps.tile([128, HID], F32, tag="mm", name="h2ps")
                        nc.tensor.matmul(h2ps[:, :],
                                         lhsT=aggT[:, dh * 128:(dh + 1) * 128],
                                         rhs=w2[:], start=True, stop=False,
                                         skip_group_check=True)
                        rb = 64 + 32 * dh
                        nc.tensor.matmul(h2ps[:, :],
                                         lhsT=pqt[rb:rb + 1, :],
                                         rhs=wrhs[rb:rb + 1, :],
                                         start=False, stop=True,
                                         skip_group_check=True,
                                         tile_position=(rb, 0))
                        if dh == 0:
                            nc.scalar.activation(h2r[:, dh, :], h2ps[:, :], RELU)
                        else:
                            nc.vector.tensor_scalar(
                                out=h2r[:, dh, :], in0=h2ps[:, :], scalar1=0.0,
                                scalar2=None, op0=mybir.AluOpType.max)
                    # readout: hg[:, gi] = sum_d h2r[d, :] * invin[d]/200
                    for dh in range(2):
                        nc.tensor.matmul(hgps[:, gi:gi + 1],
                                         lhsT=h2r[:, dh, :],
                                         rhs=invw[:, dh, gi:gi + 1],
                                         start=(dh == 0), stop=(dh == 1),
                                         skip_group_check=True)
                nc.vector.tensor_copy(hg[:, 0, :], hgps[:])
                set_dump(hg[:, 0, :], 128, GPC)

            if stage < 6 and dbg_d is not None:
                dbgt = gp.tile([128, 512], F32)
                nc.vector.memset(dbgt[:], 0.0)
                ap, h, w = dump
                w = min(512, w)
                nc.vector.tensor_copy(dbgt[0:h, 0:w], ap[:, 0:w])
                nc.sync.dma_start(dbg_d[:], dbgt[:])
                nc.sync.dma_start(out_d[:], dbgt[0:GPC, 0:NCLS])

            # ---------------- MLP head + softmax ----------------
            if stage >= 6:
                x = hg
                for li, (fi, fo) in enumerate(MLP_DIMS):
                    itiles = fi // 128
                    otiles = max(1, fo // 128)
                    m = 128 if fo >= 128 else fo
                    xn = gp.tile([128, otiles, GPC], F16, tag=f"x{li}",
                                 name=f"x{li}")
                    for ot in range(otiles):
                        ps = mmps.tile([128, GPC], F32, tag="mm", name="mlp_ps")
                        for it in range(itiles):
                            nc.tensor.matmul(
                                ps[0:m, 0:GPC],
                                lhsT=w_sb[li][:, it, ot * 128:ot * 128 + m],
                                rhs=x[:, it, :], start=(it == 0),
                                stop=(it == itiles - 1),
                                skip_group_check=True)
                        if li < len(MLP_DIMS) - 1:
                            if ot % 2 == 1:
                                nc.vector.tensor_scalar(
                                    out=xn[:, ot, :], in0=ps[:, 0:GPC],
                                    scalar1=bc_sb[li][:, ot:ot + 1],
                                    scalar2=0.0, op0=mybir.AluOpType.add,
                                    op1=mybir.AluOpType.max)
                            else:
                                nc.scalar.activation(
                                    xn[:, ot, :], ps[:, 0:GPC], RELU,
                                    bias=bc_sb[li][:, ot:ot + 1])
                        else:
                            nc.vector.tensor_scalar(
                                out=xn[0:m, ot, :], in0=ps[0:m, 0:GPC],
                                scalar1=bc_sb[li][0:m, ot:ot + 1], scalar2=None,
                                op0=mybir.AluOpType.add)
                    x = xn

                # softmax over classes: transpose [NCLS, GPC] -> [GPC, NCLS]
                x32 = gp.tile([NCLS, GPC], F32)
                nc.vector.tensor_copy(x32[:], x[0:NCLS, 0, :])
                tr_ps = mmps.tile([128, 128], F32, tag="mm", name="sm_ps")
                nc.tensor.transpose(tr_ps[0:GPC, 0:NCLS], x32[:],
                                    identf[0:NCLS, 0:NCLS])
                sm = gp.tile([GPC, NCLS], F32)
                nc.vector.tensor_copy(sm[:], tr_ps[0:GPC, 0:NCLS])
                mx = gp.tile([GPC, 1], F32)
                nc.vector.tensor_reduce(out=mx[:], in_=sm[:],
                                        axis=mybir.AxisListType.X,
                                        op=mybir.AluOpType.max)
                nc.vector.tensor_scalar(out=sm[:], in0=sm[:], scalar1=mx[:],
                                        scalar2=None,
                                        op0=mybir.AluOpType.subtract)
                nc.scalar.activation(sm[:], sm[:], EXP)
                ssum = gp.tile([GPC, 1], F32)
                nc.vector.tensor_reduce(out=ssum[:], in_=sm[:],
                                        axis=mybir.AxisListType.X,
                                        op=mybir.AluOpType.add)
                rsum = gp.tile([GPC, 1], F32)
                nc.vector.reciprocal(rsum[:], ssum[:])
                probs = gp.tile([GPC, NCLS], F32)
                nc.vector.tensor_scalar(out=probs[:], in0=sm[:], scalar1=rsum[:],
                                        scalar2=None, op0=mybir.AluOpType.mult)
                nc.sync.dma_start(out_d[:], probs[:])

    nc.compile()
    return nc


# --------------------------------------------------------------------------
# Entry point
# --------------------------------------------------------------------------

def _weights_inputs(W1, b1, W2, b2, Wa, ba, Wb, bb, Wc, bc, Wd, bd, We, be):
    wrhs = np.zeros((128, HID), np.float32)
    wrhs[0] = wrhs[32] = np.asarray(W1, np.float32).reshape(HID)
    wrhs[1] = wrhs[33] = np.asarray(b1, np.float32).reshape(HID)
    wrhs[64] = wrhs[96] = np.asarray(b2, np.float32).reshape(HID)
    base = {
        "wrhs": wrhs.astype(np.float16),
        "w2": np.ascontiguousarray(np.asarray(W2, np.float32)).astype(np.float16),
        "dmask": np.stack([np.ones(128, np.float32),
                           (np.arange(128) < 72).astype(np.float32)],
                          axis=1),
    }
    for li, (w, bvec) in enumerate(zip((Wa, Wb, Wc, Wd, We), (ba, bb, bc, bd, be))):
        w = np.asarray(w, np.float32)
        bvec = np.asarray(bvec, np.float32)
        fi, fo = w.shape
        base[f"mw{li}"] = np.ascontiguousarray(
            w.reshape(fi // 128, 128, fo).transpose(1, 0, 2)).astype(np.float16)
        if fo >= 128:
            bcol = np.ascontiguousarray(bvec.reshape(-1, 128).T)
        else:
            bcol = np.zeros((128, 1), np.float32)
            bcol[:fo, 0] = bvec
        base[f"mbc{li}"] = bcol
    return base


def kernel(src, dst, W1, b1, W2, b2, Wa, ba, Wb, bb, Wc, bc, Wd, bd, We, be,
           stage=6):
    global LAST_RESULTS
    C, edata = _preprocess(src, dst)
    key = (C, stage)
    if key not in _PROGRAM_CACHE:
        _PROGRAM_CACHE[key] = _build_program(C, stage=stage)
    nc = _PROGRAM_CACHE[key]

    base = _weights_inputs(W1, b1, W2, b2, Wa, ba, Wb, bb, Wc, bc, Wd, bd,
                           We, be)
    in_maps = [dict(base, sf=edata[c][0].reshape(128, 1, -1),
                    df=edata[c][1].reshape(128, 1, -1))
               for c in range(N_CORES)]
    LAST_RESULTS = run_bass_kernel_spmd(nc, in_maps, list(range(N_CORES)))
    out = np.concatenate([LAST_RESULTS.results[c]["out"] for c in range(N_CORES)],
                         axis=0)
    return out.astype(np.float32)
